# revision 17
# baseline (speedup 1.0000x reference)
"""Trainium2 Bass kernel for a dense transformer block (RMSNorm -> causal MHA
-> residual -> RMSNorm -> SwiGLU MLP -> residual), distributed over 8
NeuronCores with zero collectives.

Sharding: core c handles batch b = c//2 and query parity half = c%2 (the
interleaved token slice half::2, QT=1024 query tokens per core).  Each core
computes K/V for its whole batch; queries / out-proj / MLP only for its
1024 tokens.

This version runs all matmul operands in fp16 (full PE speed at any tile
size, half the DMA + SBUF of fp32), loads every weight exactly once
(weight-stationary loops over resident normalized activations), keeps the
softmax denominator off the PE (DVE accumulation of exp tiles), applies the
causal mask as a 0/1 fp16 multiply only on diagonal tiles (translation
invariance -> 8 distinct mask tiles), and uses 2-bank-wide PSUM tiles so
Act-engine instructions amortize their access latency.
"""

import numpy as np

import concourse.bass as bass
import concourse.bacc as bacc
import concourse.mybir as mybir
from concourse.tile import TileContext
from concourse.bass_utils import run_bass_kernel_spmd

F32 = mybir.dt.float32
F16 = mybir.dt.float16
AF = mybir.ActivationFunctionType
ALU = mybir.AluOpType

P = 128
N_CORES = 8
EPS = 1e-6


class CFG:
    def __init__(self, D, T, FF, QT):
        self.D, self.T, self.TD, self.FF, self.QT = D, T, D, FF, QT
        self.NS = 512
        self.DT = D // P            # contraction tiles over model dim
        self.H = self.TD // P       # heads (dh == P)
        self.KT = T // P            # key tiles
        self.NB = T // self.NS      # 512-token blocks over full sequence
        self.NQS = QT // self.NS    # query slices
        self.NVS = self.TD // self.NS  # v column slabs
        self.NFT = FF // P          # ff tiles
        self.NDCT = D // P          # output col tiles
        self.stride = T // QT       # query interleave stride
        self.NSLOT = self.stride * self.NS // P  # partial (diagonal) k tiles / slice
        self.ISQ = 1.0 / float(np.sqrt(P))

    def nkt(self, qs):
        return min((qs + 1) * self.stride * self.NS // P, self.KT)

    def kt0(self, qs):
        # first partially-masked k tile for query slice qs
        return self.stride * qs * self.NS // P


FULL = CFG(D=2048, T=2048, FF=8192, QT=1024)


def build(cfg):
    D, T, TD, FF, QT, NS = cfg.D, cfg.T, cfg.TD, cfg.FF, cfg.QT, cfg.NS
    DT, H, KT, NB, NQS = cfg.DT, cfg.H, cfg.KT, cfg.NB, cfg.NQS
    NVS, NFT, NDCT, NSLOT = cfg.NVS, cfg.NFT, cfg.NDCT, cfg.NSLOT
    HPS = NS // P               # heads per v slab
    KG = max(T // 1024, 1)      # 1024-token groups for K projection
    stride = cfg.stride

    nc = bacc.Bacc("TRN2", target_bir_lowering=False, num_devices=N_CORES)

    # ---- inputs (pre-tiled on host, fp16) ----
    x_in = nc.dram_tensor("x_in", [DT, P, T], F16, kind="ExternalInput")
    xq_in = nc.dram_tensor("xq_in", [DT, P, QT], F16, kind="ExternalInput")
    m01_in = nc.dram_tensor("m01_in", [P, NSLOT, NS], F16, kind="ExternalInput")
    wq_in = nc.dram_tensor("wq_in", [H, P, DT, P], F16, kind="ExternalInput")
    wk_in = nc.dram_tensor("wk_in", [H, P, DT, P], F16, kind="ExternalInput")
    wv_in = nc.dram_tensor("wv_in", [NVS, P, DT, NS], F16, kind="ExternalInput")
    wo_in = nc.dram_tensor("wo_in", [NDCT, P, H, P], F16, kind="ExternalInput")
    wg_in = nc.dram_tensor("wg_in", [NFT, P, DT, P], F16, kind="ExternalInput")
    wu_in = nc.dram_tensor("wu_in", [NFT, P, DT, P], F16, kind="ExternalInput")
    wd_in = nc.dram_tensor("wd_in", [NDCT, P, NFT, P], F16, kind="ExternalInput")
    y_out = nc.dram_tensor("y_out", [NDCT, P, QT], F32, kind="ExternalOutput")

    # ---- scratch DRAM (K/V spill, fp16) ----
    k_d = nc.dram_tensor("k_d", [H, P, T], F16)
    v_d = nc.dram_tensor("v_d", [NVS, KT, P, NS], F16)

    with TileContext(nc) as tc, \
            nc.allow_low_precision("fp16 softmax/norm sums; tol 2e-2"):
        pc = tc.alloc_tile_pool(name="const", bufs=1)
        ones_c = pc.tile([P, 1], F16, tag="ones_c")
        nc.vector.memset(ones_c[:], 1.0)
        ones_r = pc.tile([1, P], F16, tag="ones_r")
        nc.vector.memset(ones_r[:], 1.0)
        epsT = pc.tile([1, 1], F32, tag="eps")
        nc.vector.memset(epsT[:], EPS)
        rec_row = pc.tile([1, T], F16, tag="rec_row")
        m01 = pc.tile([P, NSLOT, NS], F16, tag="m01")
        nc.sync.dma_start(out=m01[:], in_=m01_in[:])

        # persistent activations
        pax = tc.alloc_tile_pool(name="ax", bufs=1)      # hq + xq (A -> P3)
        xq_sb = pax.tile([P, DT, QT], F16, tag="xq")
        nc.sync.dma_start(
            out=xq_sb[:], in_=xq_in.rearrange("a p c -> p a c"))
        hq = pax.tile([P, DT, QT], F16, tag="hq")

        # ================= A: rmsnorm stats + h (+hq) =================
        ph = tc.alloc_tile_pool(name="h", bufs=1)        # h blocks (A -> B)
        h_blk = [ph.tile([P, DT, NS], F16, tag=f"h{tb}", name=f"h{tb}")
                 for tb in range(NB)]
        with nc.named_scope("A"):
            with tc.tile_pool(name="pa", bufs=2) as pa, \
                 tc.tile_pool(name="pa_ps", bufs=2, space="PSUM") as pa_ps, \
                 tc.tile_pool(name="pa_bc", bufs=2, space="PSUM") as pa_bc:
                for tb in range(NB):
                    t0 = tb * NS
                    xb = pa.tile([P, DT, NS], F16, tag="xb")
                    nc.sync.dma_start(
                        out=xb[:],
                        in_=x_in[:, :, t0:t0 + NS].rearrange("a p c -> p a c"))
                    ssp = pa_ps.tile([1, NS], F32, tag="ssp")
                    for c in range(DT // 4):
                        sq = pa.tile([P, 4, NS], F16, tag="sq")
                        nc.scalar.activation(sq[:], xb[:, 4 * c:4 * c + 4, :],
                                             AF.Square)
                        for j in range(4):
                            dt = 4 * c + j
                            nc.tensor.matmul(ssp[:], ones_c[:], sq[:, j, :],
                                             start=(dt == 0),
                                             stop=(dt == DT - 1))
                    srow = pa.tile([1, NS], F32, tag="srow")
                    nc.scalar.activation(srow[:], ssp[:], AF.Sqrt,
                                         scale=1.0 / D, bias=epsT[:])
                    nc.vector.reciprocal(rec_row[:, t0:t0 + NS], srow[:])
                    bcp = pa_bc.tile([P, NS], F32, tag="bcp")
                    nc.tensor.matmul(bcp[:], ones_r[:],
                                     rec_row[:, t0:t0 + NS],
                                     start=True, stop=True)
                    bcs = pa.tile([P, NS], F16, tag="bcs")
                    nc.scalar.copy(bcs[:], bcp[:])
                    for dt in range(DT):
                        nc.vector.tensor_tensor(h_blk[tb][:, dt, :],
                                                xb[:, dt, :], bcs[:], ALU.mult)
                # hq = xq * rsqrt: recompute stats from the (host-sliced)
                # query tokens — keeps the program parity-independent.
                for ws in range(NQS):
                    q0 = ws * NS
                    sspq = pa_ps.tile([1, NS], F32, tag="ssp")
                    for c in range(DT // 4):
                        sqq = pa.tile([P, 4, NS], F16, tag="sq")
                        nc.scalar.activation(
                            sqq[:], xq_sb[:, 4 * c:4 * c + 4, q0:q0 + NS],
                            AF.Square)
                        for j in range(4):
                            dt = 4 * c + j
                            nc.tensor.matmul(sspq[:], ones_c[:], sqq[:, j, :],
                                             start=(dt == 0),
                                             stop=(dt == DT - 1))
                    srowq = pa.tile([1, NS], F32, tag="srow")
                    nc.scalar.activation(srowq[:], sspq[:], AF.Sqrt,
                                         scale=1.0 / D, bias=epsT[:])
                    recq = pa.tile([1, NS], F16, tag="recq")
                    nc.vector.reciprocal(recq[:], srowq[:])
                    bcq = pa_bc.tile([P, NS], F32, tag="bcp")
                    nc.tensor.matmul(bcq[:], ones_r[:], recq[:],
                                     start=True, stop=True)
                    bcqs = pa.tile([P, NS], F16, tag="bcs")
                    nc.scalar.copy(bcqs[:], bcq[:])
                    for dt in range(DT):
                        nc.vector.tensor_tensor(hq[:, dt, q0:q0 + NS],
                                                xq_sb[:, dt, q0:q0 + NS],
                                                bcqs[:], ALU.mult)

        # ================= B: K and V projections (spill to DRAM) ========
        with nc.named_scope("B"):
            with tc.tile_pool(name="pb", bufs=2) as pb, \
                 tc.tile_pool(name="pbw", bufs=2) as pbw, \
                 tc.tile_pool(name="pb_k", bufs=2, space="PSUM") as pb_k, \
                 tc.tile_pool(name="pb_v", bufs=2, space="PSUM") as pb_v:
                for hh in range(H):
                    wk = pbw.tile([P, DT, P], F16, tag="wk")
                    nc.sync.dma_start(out=wk[:], in_=wk_in[hh])
                    for g in range(KG):
                        kps = pb_k.tile([P, 2 * NS], F32, tag="kps")
                        for half in range(2):
                            sl = g * 2 + half
                            for dt in range(DT):
                                nc.tensor.matmul(
                                    kps[:, half * NS:(half + 1) * NS],
                                    wk[:, dt, :], h_blk[sl][:, dt, :],
                                    start=(dt == 0), stop=(dt == DT - 1))
                        kcp = pb.tile([P, 2 * NS], F16, tag="kcp")
                        nc.scalar.copy(kcp[:], kps[:])
                        nc.sync.dma_start(
                            out=k_d[hh][:, g * 2 * NS:(g + 1) * 2 * NS],
                            in_=kcp[:])
                for vs in range(NVS):
                    wv = pbw.tile([P, DT, NS], F16, tag="wv")
                    nc.sync.dma_start(out=wv[:], in_=wv_in[vs])
                    for kt in range(KT):
                        tb, off = divmod(kt * P, NS)
                        vps = pb_v.tile([P, NS], F32, tag="vps")
                        for dt in range(DT):
                            nc.tensor.matmul(
                                vps[:], h_blk[tb][:, dt, off:off + P],
                                wv[:, dt, :],
                                start=(dt == 0), stop=(dt == DT - 1))
                        vcp = pb.tile([P, NS], F16, tag="vcp")
                        nc.scalar.copy(vcp[:], vps[:])
                        nc.sync.dma_start(out=v_d[vs, kt], in_=vcp[:])
        ph.release()

        # ================= P2: Q projection + causal attention ===========
        po = tc.alloc_tile_pool(name="o", bufs=1)        # o (P2 -> P3)
        o_w = [po.tile([P, H, NS], F16, tag=f"o{ws}", name=f"o{ws}")
               for ws in range(NQS)]
        with nc.named_scope("P2"):
            with tc.tile_pool(name="p2", bufs=2) as p2, \
                 tc.tile_pool(name="p2kv", bufs=2) as p2kv, \
                 tc.tile_pool(name="p2pex", bufs=3) as p2pex, \
                 tc.tile_pool(name="p2w", bufs=2) as p2w, \
                 tc.tile_pool(name="p2mm", bufs=2, space="PSUM") as p2mm, \
                 tc.tile_pool(name="p2acc", bufs=2, space="PSUM") as p2acc, \
                 tc.tile_pool(name="p2s", bufs=1, space="PSUM") as p2s, \
                 tc.tile_pool(name="p2bc", bufs=1, space="PSUM") as p2bc:
                for hh in range(H):
                    kh = p2kv.tile([P, T], F16, tag="kh")
                    nc.sync.dma_start(out=kh[:], in_=k_d[hh])
                    vh = p2kv.tile([P, KT, P], F16, tag="vh")
                    voff = (hh % HPS) * P
                    nc.sync.dma_start(
                        out=vh[:],
                        in_=v_d[hh // HPS].rearrange(
                            "t p c -> p t c")[:, :, voff:voff + P])
                    # --- Q projection for this head (fills PE while Act
                    #     runs exp for the previous head) ---
                    wq = p2w.tile([P, DT, P], F16, tag="wq")
                    nc.sync.dma_start(out=wq[:], in_=wq_in[hh])
                    qps = p2mm.tile([P, NQS * NS], F32, tag="mm")
                    for ws in range(NQS):
                        for dt in range(DT):
                            nc.tensor.matmul(
                                qps[:, ws * NS:(ws + 1) * NS],
                                wq[:, dt, :], hq[:, dt, ws * NS:(ws + 1) * NS],
                                start=(dt == 0), stop=(dt == DT - 1))
                    qh = p2.tile([P, QT], F16, tag="qh")
                    nc.scalar.copy(qh[:], qps[:])
                    for qs in range(NQS):
                        nkt = cfg.nkt(qs)
                        kt0 = cfg.kt0(qs)
                        npair = nkt // 2
                        oacc = p2acc.tile([P, NS], F32, tag="oacc")
                        dsum = p2.tile([P, NS], F16, tag="dsum")
                        scps = {}

                        def emit_scp(kp):
                            scp = p2mm.tile([P, 2, NS], F32, tag="mm")
                            for half in range(2):
                                kt = 2 * kp + half
                                nc.tensor.matmul(
                                    scp[:, half, :], kh[:, kt * P:(kt + 1) * P],
                                    qh[:, qs * NS:(qs + 1) * NS],
                                    start=True, stop=True)
                            scps[kp] = scp

                        def emit_rest(kp):
                            scp = scps.pop(kp)
                            pex = p2pex.tile([P, 2, NS], F16, tag="pex")
                            nc.scalar.activation(pex[:], scp[:], AF.Exp,
                                                 scale=cfg.ISQ)
                            if 2 * kp >= kt0:
                                s = 2 * kp - kt0
                                pexm = p2pex.tile([P, 2, NS], F16, tag="pexm")
                                nc.vector.tensor_tensor(
                                    pexm[:], pex[:], m01[:, s:s + 2, :],
                                    ALU.mult)
                                pex = pexm
                            if kp == 0:
                                nc.vector.tensor_tensor(
                                    dsum[:], pex[:, 0, :], pex[:, 1, :],
                                    ALU.add)
                            else:
                                for half in range(2):
                                    nc.vector.tensor_tensor(
                                        dsum[:], dsum[:], pex[:, half, :],
                                        ALU.add)
                            for half in range(2):
                                kt = 2 * kp + half
                                nc.tensor.matmul(
                                    oacc[:], vh[:, kt, :], pex[:, half, :],
                                    start=(kt == 0), stop=(kt == nkt - 1))

                        emit_scp(0)
                        for kp in range(1, npair):
                            emit_scp(kp)
                            emit_rest(kp - 1)
                        emit_rest(npair - 1)
                        # softmax denominator -> normalize
                        dps = p2s.tile([1, NS], F32, tag="dps")
                        nc.tensor.matmul(dps[:], ones_c[:], dsum[:],
                                         start=True, stop=True)
                        recd = p2.tile([1, NS], F16, tag="recd")
                        nc.vector.reciprocal(recd[:], dps[:])
                        bcd = p2bc.tile([P, NS], F32, tag="bcd")
                        nc.tensor.matmul(bcd[:], ones_r[:], recd[:],
                                         start=True, stop=True)
                        bcds = p2.tile([P, NS], F16, tag="bcds")
                        nc.scalar.copy(bcds[:], bcd[:])
                        nc.vector.tensor_tensor(o_w[qs][:, hh, :], oacc[:],
                                                bcds[:], ALU.mult)

        # ================= P3: out-proj + residual + norm2 ===============
        # right-side stack: lifetime (P3 -> P5) crosses po's release
        px2 = tc.alloc_tile_pool(name="x2h2", bufs=1, side="right")
        x2 = px2.tile([P, NDCT, QT], F16, tag="x2")
        h2 = px2.tile([P, DT, QT], F16, tag="h2")
        with nc.named_scope("P3"):
            with tc.tile_pool(name="p3", bufs=2) as p3, \
                 tc.tile_pool(name="p3w", bufs=2) as p3w, \
                 tc.tile_pool(name="p3mm", bufs=2, space="PSUM") as p3mm, \
                 tc.tile_pool(name="p3s", bufs=1, space="PSUM") as p3s, \
                 tc.tile_pool(name="p3bc", bufs=1, space="PSUM") as p3bc:
                ssp2 = p3s.tile([1, QT], F32, tag="ssp2")
                for dct in range(NDCT):
                    wo = p3w.tile([P, H, P], F16, tag="wo")
                    nc.sync.dma_start(out=wo[:], in_=wo_in[dct])
                    ops = p3mm.tile([P, NQS, NS], F32, tag="ops")
                    for ws in range(NQS):
                        for hh in range(H):
                            nc.tensor.matmul(
                                ops[:, ws, :], wo[:, hh, :], o_w[ws][:, hh, :],
                                start=(hh == 0), stop=(hh == H - 1))
                    nc.vector.tensor_tensor(
                        x2[:, dct, :],
                        ops.rearrange("p a b -> p (a b)"),
                        xq_sb[:, dct, :], ALU.add)
                    sq2 = p3.tile([P, QT], F16, tag="sq2")
                    nc.scalar.activation(sq2[:], x2[:, dct, :], AF.Square)
                    for ws in range(NQS):
                        nc.tensor.matmul(ssp2[:, ws * NS:(ws + 1) * NS],
                                         ones_c[:],
                                         sq2[:, ws * NS:(ws + 1) * NS],
                                         start=(dct == 0),
                                         stop=(dct == NDCT - 1))
                srow2 = p3.tile([1, QT], F32, tag="srow2")
                nc.scalar.activation(srow2[:], ssp2[:], AF.Sqrt,
                                     scale=1.0 / D, bias=epsT[:])
                rec2 = p3.tile([1, QT], F16, tag="rec2")
                nc.vector.reciprocal(rec2[:], srow2[:])
                bc2 = p3bc.tile([P, QT], F32, tag="bc2")
                for ws in range(NQS):
                    nc.tensor.matmul(bc2[:, ws * NS:(ws + 1) * NS], ones_r[:],
                                     rec2[:, ws * NS:(ws + 1) * NS],
                                     start=True, stop=True)
                bc2s = p3.tile([P, QT], F16, tag="bc2s", bufs=1)
                nc.scalar.copy(bc2s[:], bc2[:])
                for dt in range(DT):
                    nc.vector.tensor_tensor(h2[:, dt, :], x2[:, dt, :],
                                            bc2s[:], ALU.mult)
        po.release()
        pax.release()

        # ================= P5: SwiGLU MLP + residual =====================
        with nc.named_scope("P5"):
            with tc.tile_pool(name="p5", bufs=2) as p5, \
                 tc.tile_pool(name="p5w", bufs=2) as p5w, \
                 tc.tile_pool(name="p5mt", bufs=1) as p5mt, \
                 tc.tile_pool(name="p5gu", bufs=2, space="PSUM") as p5gu, \
                 tc.tile_pool(name="p5d", bufs=3, space="PSUM") as p5d:
                for ws in range(NQS):
                    q0 = ws * NS
                    mt = p5mt.tile([P, NFT, NS], F16, tag="mt")
                    for ft in range(NFT):
                        wg = p5w.tile([P, DT, P], F16, tag="wg")
                        nc.sync.dma_start(out=wg[:], in_=wg_in[ft])
                        wu = p5w.tile([P, DT, P], F16, tag="wu")
                        nc.sync.dma_start(out=wu[:], in_=wu_in[ft])
                        guw = p5gu.tile([P, 2, NS], F32, tag="guw")
                        for dt in range(DT):
                            nc.tensor.matmul(
                                guw[:, 0, :], wg[:, dt, :],
                                h2[:, dt, q0:q0 + NS],
                                start=(dt == 0), stop=(dt == DT - 1))
                        for dt in range(DT):
                            nc.tensor.matmul(
                                guw[:, 1, :], wu[:, dt, :],
                                h2[:, dt, q0:q0 + NS],
                                start=(dt == 0), stop=(dt == DT - 1))
                        sg = p5.tile([P, NS], F16, tag="sg")
                        nc.scalar.activation(sg[:], guw[:, 0, :], AF.Silu)
                        nc.vector.tensor_tensor(mt[:, ft, :], sg[:],
                                                guw[:, 1, :], ALU.mult)
                    for dct in range(NDCT):
                        wd = p5w.tile([P, NFT, P], F16, tag="wd")
                        nc.sync.dma_start(out=wd[:], in_=wd_in[dct])
                        dps = p5d.tile([P, NS], F32, tag="dacc")
                        for ft in range(NFT):
                            nc.tensor.matmul(dps[:], wd[:, ft, :],
                                             mt[:, ft, :],
                                             start=(ft == 0),
                                             stop=(ft == NFT - 1))
                        yt = p5.tile([P, NS], F32, tag="yt")
                        nc.vector.tensor_tensor(yt[:], dps[:],
                                                x2[:, dct, q0:q0 + NS],
                                                ALU.add)
                        nc.sync.dma_start(out=y_out[dct][:, q0:q0 + NS],
                                          in_=yt[:])
        px2.release()
        pc.release()

    nc.compile()
    return nc


# --------------------------------------------------------------------------
# Host side
# --------------------------------------------------------------------------

_NC_CACHE = {}


def _get_nc(cfg):
    key = (cfg.D, cfg.T, cfg.FF, cfg.QT)
    if key not in _NC_CACHE:
        _NC_CACHE[key] = build(cfg)
    return _NC_CACHE[key]


def _tile_lhs(a, ncols):
    # [Din, Cout] -> [Cout/ncols, P, Din/P, ncols]
    d, c = a.shape
    return np.ascontiguousarray(
        a.reshape(d // P, P, c // ncols, ncols).transpose(2, 1, 0, 3))


def prep_weights(cfg, w_qkv, w_out, w_gate, w_up, w_down, ln1, ln2):
    D, TD, FF, NS = cfg.D, cfg.TD, cfg.FF, cfg.NS
    f32, f16 = np.float32, np.float16
    w_qkv_f = (np.asarray(w_qkv, f32) * np.asarray(ln1, f32)[None, :])
    wqT = w_qkv_f[0:TD].T
    wkT = w_qkv_f[TD:2 * TD].T
    wvT = w_qkv_f[2 * TD:3 * TD].T
    woT = np.asarray(w_out, f32).T            # [TD, D]
    wgT = (np.asarray(w_gate, f32) * np.asarray(ln2, f32)[None, :]).T
    wuT = (np.asarray(w_up, f32) * np.asarray(ln2, f32)[None, :]).T
    wdT = np.asarray(w_down, f32).T           # [FF, D]

    wd_in = np.ascontiguousarray(
        wdT.reshape(cfg.NFT, P, cfg.NDCT, P).transpose(2, 1, 0, 3))
    return dict(
        wq_in=_tile_lhs(wqT, P).astype(f16),
        wk_in=_tile_lhs(wkT, P).astype(f16),
        wv_in=_tile_lhs(wvT, NS).astype(f16),
        wo_in=_tile_lhs(woT, P).astype(f16),
        wg_in=_tile_lhs(wgT, P).astype(f16),
        wu_in=_tile_lhs(wuT, P).astype(f16),
        wd_in=wd_in.astype(f16),
    )


def prep_core_inputs(cfg, xb, parity, wdict):
    """Per-core tensors for batch slice xb [T, D]; query tokens are the
    interleaved slice parity::stride."""
    T, D, QT, NS = cfg.T, cfg.D, cfg.QT, cfg.NS
    stride = cfg.stride
    f16 = np.float16
    xT = np.ascontiguousarray(np.asarray(xb, np.float32).T)   # [D, T]
    x_in = xT.reshape(cfg.DT, P, T).astype(f16)
    xq_in = np.ascontiguousarray(
        xT[:, parity::stride]).reshape(cfg.DT, P, QT).astype(f16)
    # 0/1 mask for the NSLOT diagonal k tiles of every query slice:
    # slot s, row ki, col q allowed iff 128*s + ki <= stride*q + parity
    ki = np.arange(P)[:, None, None]
    s = np.arange(cfg.NSLOT)[None, :, None]
    q = np.arange(NS)[None, None, :]
    m01_in = ((P * s + ki) <= (stride * q + parity)).astype(f16)
    out = dict(x_in=x_in, xq_in=xq_in, m01_in=m01_in)
    out.update(wdict)
    return out


def run(cfg, x, w_qkv, w_out, w_gate, w_up, w_down, ln1, ln2):
    nc = _get_nc(cfg)
    wdict = prep_weights(cfg, w_qkv, w_out, w_gate, w_up, w_down, ln1, ln2)
    x = np.asarray(x, np.float32)
    Bc = x.shape[0]
    in_maps = []
    for c in range(N_CORES):
        b, half = divmod(c, 2)
        in_maps.append(prep_core_inputs(cfg, x[b % Bc], half, wdict))
    res = run_bass_kernel_spmd(nc, in_maps, list(range(N_CORES)))
    y = np.empty((Bc, cfg.T, cfg.D), np.float32)
    for c in range(N_CORES):
        b, parity = divmod(c, 2)
        if b < Bc:
            yc = res.results[c]["y_out"].reshape(cfg.D, cfg.QT)
            y[b, parity::cfg.stride, :] = yc.T
    return y


def kernel(x, w_qkv, w_out, w_gate, w_up, w_down, ln1, ln2):
    return run(FULL, x, w_qkv, w_out, w_gate, w_up, w_down, ln1, ln2)


# revision 23
# speedup vs baseline: 1.1689x; 1.1689x over previous
"""Trainium2 Bass kernel for a dense transformer block (RMSNorm -> causal MHA
-> residual -> RMSNorm -> SwiGLU MLP -> residual), distributed over 8
NeuronCores with zero collectives.

Sharding: core c handles batch b = c//2 and query parity half = c%2 (the
interleaved token slice half::2, QT=1024 query tokens per core).  Each core
computes K/V for its whole batch; queries / out-proj / MLP only for its
1024 tokens.

This version runs all matmul operands in fp16 (full PE speed at any tile
size, half the DMA + SBUF of fp32), loads every weight exactly once
(weight-stationary loops over resident normalized activations), keeps the
softmax denominator off the PE (DVE accumulation of exp tiles), applies the
causal mask as a 0/1 fp16 multiply only on diagonal tiles (translation
invariance -> 8 distinct mask tiles), and uses 2-bank-wide PSUM tiles so
Act-engine instructions amortize their access latency.
"""

import numpy as np

import concourse.bass as bass
import concourse.bass_isa as bass_isa
import concourse.bacc as bacc
import concourse.mybir as mybir
from concourse.tile import TileContext
from concourse.bass_utils import run_bass_kernel_spmd

F32 = mybir.dt.float32
F16 = mybir.dt.float16
AF = mybir.ActivationFunctionType
ALU = mybir.AluOpType

P = 128
N_CORES = 8
EPS = 1e-6


class CFG:
    def __init__(self, D, T, FF, QT):
        self.D, self.T, self.TD, self.FF, self.QT = D, T, D, FF, QT
        self.NS = 512
        self.DT = D // P            # contraction tiles over model dim
        self.H = self.TD // P       # heads (dh == P)
        self.KT = T // P            # key tiles
        self.NB = T // self.NS      # 512-token blocks over full sequence
        self.NQS = QT // self.NS    # query slices
        self.NVS = self.TD // self.NS  # v column slabs
        self.NFT = FF // P          # ff tiles
        self.NDCT = D // P          # output col tiles
        self.stride = T // QT       # query interleave stride
        self.NSLOT = self.stride * self.NS // P  # partial (diagonal) k tiles / slice
        self.ISQ = 1.0 / float(np.sqrt(P))

    def nkt(self, qs):
        return min((qs + 1) * self.stride * self.NS // P, self.KT)

    def kt0(self, qs):
        # first partially-masked k tile for query slice qs
        return self.stride * qs * self.NS // P


FULL = CFG(D=2048, T=2048, FF=8192, QT=1024)


def build(cfg):
    D, T, TD, FF, QT, NS = cfg.D, cfg.T, cfg.TD, cfg.FF, cfg.QT, cfg.NS
    DT, H, KT, NB, NQS = cfg.DT, cfg.H, cfg.KT, cfg.NB, cfg.NQS
    NVS, NFT, NDCT, NSLOT = cfg.NVS, cfg.NFT, cfg.NDCT, cfg.NSLOT
    HPS = NS // P               # heads per v slab
    KG = max(T // 1024, 1)      # 1024-token groups for K projection
    stride = cfg.stride

    nc = bacc.Bacc("TRN2", target_bir_lowering=False, num_devices=N_CORES)

    # ---- inputs (pre-tiled on host, fp16) ----
    x_in = nc.dram_tensor("x_in", [DT, P, T], F16, kind="ExternalInput")
    xq_in = nc.dram_tensor("xq_in", [DT, P, QT], F16, kind="ExternalInput")
    m01_in = nc.dram_tensor("m01_in", [P, NSLOT, NS], F16, kind="ExternalInput")
    wq_in = nc.dram_tensor("wq_in", [H, P, DT, P], F16, kind="ExternalInput")
    wk_in = nc.dram_tensor("wk_in", [H, P, DT, P], F16, kind="ExternalInput")
    wv_in = nc.dram_tensor("wv_in", [NVS, P, DT, NS], F16, kind="ExternalInput")
    wo_in = nc.dram_tensor("wo_in", [NDCT, P, H, P], F16, kind="ExternalInput")
    wg_in = nc.dram_tensor("wg_in", [NFT, P, DT, P], F16, kind="ExternalInput")
    wu_in = nc.dram_tensor("wu_in", [NFT, P, DT, P], F16, kind="ExternalInput")
    wd_in = nc.dram_tensor("wd_in", [NDCT, P, NFT, P], F16, kind="ExternalInput")
    y_out = nc.dram_tensor("y_out", [NDCT, P, QT], F32, kind="ExternalOutput")

    # ---- scratch DRAM (K/V spill, fp16) ----
    k_d = nc.dram_tensor("k_d", [H, P, T], F16)
    v_d = nc.dram_tensor("v_d", [NVS, KT, P, NS], F16)

    with TileContext(nc) as tc, \
            nc.allow_low_precision("fp16 softmax/norm sums; tol 2e-2"):
        pc = tc.alloc_tile_pool(name="const", bufs=1)
        ones_c = pc.tile([P, 1], F16, tag="ones_c")
        nc.vector.memset(ones_c[:], 1.0)
        ones_r = pc.tile([1, P], F16, tag="ones_r")
        nc.vector.memset(ones_r[:], 1.0)
        epsT = pc.tile([1, 1], F32, tag="eps")
        nc.vector.memset(epsT[:], EPS)
        rec_row = pc.tile([1, T], F16, tag="rec_row")
        m01 = pc.tile([P, NSLOT, NS], F16, tag="m01")
        nc.sync.dma_start(out=m01[:], in_=m01_in[:])

        # persistent activations
        pax = tc.alloc_tile_pool(name="ax", bufs=1)      # hq + xq (A -> P3)
        xq_sb = pax.tile([P, DT, QT], F16, tag="xq")
        nc.sync.dma_start(
            out=xq_sb[:], in_=xq_in.rearrange("a p c -> p a c"))
        hq = pax.tile([P, DT, QT], F16, tag="hq")

        # ================= A: rmsnorm stats + h (+hq) =================
        ph = tc.alloc_tile_pool(name="h", bufs=1)        # h blocks (A -> B)
        h_blk = [ph.tile([P, DT, NS], F16, tag=f"h{tb}", name=f"h{tb}")
                 for tb in range(NB)]
        with nc.named_scope("A"):
            with tc.tile_pool(name="pa", bufs=2) as pa, \
                 tc.tile_pool(name="pa_ps", bufs=2, space="PSUM") as pa_ps, \
                 tc.tile_pool(name="pa_bc", bufs=2, space="PSUM") as pa_bc:
                for tb in range(NB):
                    t0 = tb * NS
                    xb = pa.tile([P, DT, NS], F16, tag="xb")
                    nc.sync.dma_start(
                        out=xb[:],
                        in_=x_in[:, :, t0:t0 + NS].rearrange("a p c -> p a c"))
                    ssp = pa_ps.tile([1, NS], F32, tag="ssp")
                    for c in range(DT // 4):
                        sq = pa.tile([P, 4, NS], F16, tag="sq")
                        nc.scalar.activation(sq[:], xb[:, 4 * c:4 * c + 4, :],
                                             AF.Square)
                        for j in range(4):
                            dt = 4 * c + j
                            nc.tensor.matmul(ssp[:], ones_c[:], sq[:, j, :],
                                             start=(dt == 0),
                                             stop=(dt == DT - 1))
                    srow = pa.tile([1, NS], F32, tag="srow")
                    nc.scalar.activation(srow[:], ssp[:], AF.Sqrt,
                                         scale=1.0 / D, bias=epsT[:])
                    nc.vector.reciprocal(rec_row[:, t0:t0 + NS], srow[:])
                    bcp = pa_bc.tile([P, NS], F32, tag="bcp")
                    nc.tensor.matmul(bcp[:], ones_r[:],
                                     rec_row[:, t0:t0 + NS],
                                     start=True, stop=True)
                    bcs = pa.tile([P, NS], F16, tag="bcs")
                    nc.scalar.copy(bcs[:], bcp[:])
                    for dt in range(DT):
                        nc.vector.tensor_tensor(h_blk[tb][:, dt, :],
                                                xb[:, dt, :], bcs[:], ALU.mult)
                # hq = xq * rsqrt: recompute stats from the (host-sliced)
                # query tokens — keeps the program parity-independent.
                for ws in range(NQS):
                    q0 = ws * NS
                    sspq = pa_ps.tile([1, NS], F32, tag="ssp")
                    for c in range(DT // 4):
                        sqq = pa.tile([P, 4, NS], F16, tag="sq")
                        nc.scalar.activation(
                            sqq[:], xq_sb[:, 4 * c:4 * c + 4, q0:q0 + NS],
                            AF.Square)
                        for j in range(4):
                            dt = 4 * c + j
                            nc.tensor.matmul(sspq[:], ones_c[:], sqq[:, j, :],
                                             start=(dt == 0),
                                             stop=(dt == DT - 1))
                    srowq = pa.tile([1, NS], F32, tag="srow")
                    nc.scalar.activation(srowq[:], sspq[:], AF.Sqrt,
                                         scale=1.0 / D, bias=epsT[:])
                    recq = pa.tile([1, NS], F16, tag="recq")
                    nc.vector.reciprocal(recq[:], srowq[:])
                    bcq = pa_bc.tile([P, NS], F32, tag="bcp")
                    nc.tensor.matmul(bcq[:], ones_r[:], recq[:],
                                     start=True, stop=True)
                    bcqs = pa.tile([P, NS], F16, tag="bcs")
                    nc.scalar.copy(bcqs[:], bcq[:])
                    for dt in range(DT):
                        nc.vector.tensor_tensor(hq[:, dt, q0:q0 + NS],
                                                xq_sb[:, dt, q0:q0 + NS],
                                                bcqs[:], ALU.mult)

        # ================= B: K and V projections (spill to DRAM) ========
        # K runs slice-outer so the first slice only waits on h block 0 —
        # phase A's tail hides under B's matmul stream (wk reloaded per
        # slice; the extra DMA is free next to B's PE time).
        with nc.named_scope("B"):
            with tc.tile_pool(name="pb", bufs=2) as pb, \
                 tc.tile_pool(name="pbw", bufs=2) as pbw, \
                 tc.tile_pool(name="pb_k", bufs=2, space="PSUM") as pb_k, \
                 tc.tile_pool(name="pb_v", bufs=2, space="PSUM") as pb_v:
                for sl in range(NB):
                    for hh in range(H):
                        wk = pbw.tile([P, DT, P], F16, tag="wk")
                        nc.sync.dma_start(out=wk[:], in_=wk_in[hh])
                        kps = pb_k.tile([P, NS], F32, tag="kps")
                        for dt in range(DT):
                            nc.tensor.matmul(
                                kps[:], wk[:, dt, :], h_blk[sl][:, dt, :],
                                start=(dt == 0), stop=(dt == DT - 1))
                        kcp = pb.tile([P, NS], F16, tag="kcp")
                        nc.scalar.copy(kcp[:], kps[:])
                        nc.sync.dma_start(
                            out=k_d[hh][:, sl * NS:(sl + 1) * NS],
                            in_=kcp[:])
                for vs in range(NVS):
                    wv = pbw.tile([P, DT, NS], F16, tag="wv")
                    nc.sync.dma_start(out=wv[:], in_=wv_in[vs])
                    for kt in range(KT):
                        tb, off = divmod(kt * P, NS)
                        vps = pb_v.tile([P, NS], F32, tag="vps")
                        for dt in range(DT):
                            nc.tensor.matmul(
                                vps[:], h_blk[tb][:, dt, off:off + P],
                                wv[:, dt, :],
                                start=(dt == 0), stop=(dt == DT - 1))
                        vcp = pb.tile([P, NS], F16, tag="vcp")
                        nc.scalar.copy(vcp[:], vps[:])
                        nc.sync.dma_start(out=v_d[vs, kt], in_=vcp[:])
        ph.release()

        # ================= P2: Q projection + causal attention ===========
        po = tc.alloc_tile_pool(name="o", bufs=1)        # o (P2 -> P3)
        o_w = [po.tile([P, H, NS], F16, tag=f"o{ws}", name=f"o{ws}")
               for ws in range(NQS)]
        with nc.named_scope("P2"):
            with tc.tile_pool(name="p2", bufs=2) as p2, \
                 tc.tile_pool(name="p2kv", bufs=2) as p2kv, \
                 tc.tile_pool(name="p2pex", bufs=4) as p2pex, \
                 tc.tile_pool(name="p2w", bufs=2) as p2w, \
                 tc.tile_pool(name="p2mm", bufs=3, space="PSUM") as p2mm, \
                 tc.tile_pool(name="p2acc", bufs=2, space="PSUM") as p2acc:
                for hh in range(H):
                    kh = p2kv.tile([P, T], F16, tag="kh")
                    nc.sync.dma_start(out=kh[:], in_=k_d[hh])
                    vh = p2kv.tile([P, KT, P], F16, tag="vh")
                    voff = (hh % HPS) * P
                    nc.sync.dma_start(
                        out=vh[:],
                        in_=v_d[hh // HPS].rearrange(
                            "t p c -> p t c")[:, :, voff:voff + P])
                    # --- Q projection for this head (fills PE while Act
                    #     runs exp for the previous head) ---
                    wq = p2w.tile([P, DT, P], F16, tag="wq")
                    nc.sync.dma_start(out=wq[:], in_=wq_in[hh])
                    qps = p2mm.tile([P, NQS * NS], F32, tag="mm")
                    for ws in range(NQS):
                        for dt in range(DT):
                            nc.tensor.matmul(
                                qps[:, ws * NS:(ws + 1) * NS],
                                wq[:, dt, :], hq[:, dt, ws * NS:(ws + 1) * NS],
                                start=(dt == 0), stop=(dt == DT - 1))
                    qh = p2.tile([P, QT], F16, tag="qh")
                    nc.scalar.copy(qh[:], qps[:])
                    for qs in range(NQS):
                        nkt = cfg.nkt(qs)
                        kt0 = cfg.kt0(qs)
                        npair = nkt // 2
                        oacc = p2acc.tile([P, NS], F32, tag="oacc")
                        dsum = p2.tile([P, NS], F16, tag="dsum")
                        scps = {}

                        def emit_scp(kp):
                            scp = p2mm.tile([P, 2, NS], F32, tag="mm")
                            for half in range(2):
                                kt = 2 * kp + half
                                nc.tensor.matmul(
                                    scp[:, half, :], kh[:, kt * P:(kt + 1) * P],
                                    qh[:, qs * NS:(qs + 1) * NS],
                                    start=True, stop=True)
                            scps[kp] = scp

                        def emit_rest(kp):
                            scp = scps.pop(kp)
                            pex = p2pex.tile([P, 2, NS], F16, tag="pex")
                            nc.scalar.activation(pex[:], scp[:], AF.Exp,
                                                 scale=cfg.ISQ)
                            if 2 * kp >= kt0:
                                s = 2 * kp - kt0
                                pexm = p2pex.tile([P, 2, NS], F16, tag="pexm")
                                nc.vector.tensor_tensor(
                                    pexm[:], pex[:], m01[:, s:s + 2, :],
                                    ALU.mult)
                                pex = pexm
                            if kp == 0:
                                nc.vector.tensor_tensor(
                                    dsum[:], pex[:, 0, :], pex[:, 1, :],
                                    ALU.add)
                            else:
                                for half in range(2):
                                    nc.vector.tensor_tensor(
                                        dsum[:], dsum[:], pex[:, half, :],
                                        ALU.add)
                            for half in range(2):
                                kt = 2 * kp + half
                                nc.tensor.matmul(
                                    oacc[:], vh[:, kt, :], pex[:, half, :],
                                    start=(kt == 0), stop=(kt == nkt - 1))

                        emit_scp(0)
                        if npair > 1:
                            emit_scp(1)
                        for kp in range(npair):
                            if kp + 2 < npair:
                                emit_scp(kp + 2)
                            emit_rest(kp)
                        # softmax denominator (Pool engine: cross-partition
                        # sum broadcast to every partition) -> normalize
                        dbc = p2.tile([P, NS], F16, tag="dbc")
                        nc.gpsimd.partition_all_reduce(
                            dbc[:], dsum[:], P, bass_isa.ReduceOp.add)
                        recb = p2.tile([P, NS], F16, tag="recb")
                        nc.vector.reciprocal(recb[:], dbc[:])
                        nc.vector.tensor_tensor(o_w[qs][:, hh, :], oacc[:],
                                                recb[:], ALU.mult)

        # ================= P3: out-proj + residual + norm2 ===============
        # right-side stack: lifetime (P3 -> P5) crosses po's release
        px2 = tc.alloc_tile_pool(name="x2h2", bufs=1, side="right")
        x2 = px2.tile([P, NDCT, QT], F16, tag="x2")
        h2 = px2.tile([P, DT, QT], F16, tag="h2")
        with nc.named_scope("P3"):
            with tc.tile_pool(name="p3", bufs=2) as p3, \
                 tc.tile_pool(name="p3w", bufs=2) as p3w, \
                 tc.tile_pool(name="p3mm", bufs=2, space="PSUM") as p3mm, \
                 tc.tile_pool(name="p3s", bufs=1, space="PSUM") as p3s, \
                 tc.tile_pool(name="p3bc", bufs=2, space="PSUM") as p3bc:
                ssp2 = p3s.tile([1, QT], F32, tag="ssp2")
                for dct in range(NDCT):
                    wo = p3w.tile([P, H, P], F16, tag="wo")
                    nc.sync.dma_start(out=wo[:], in_=wo_in[dct])
                    ops = p3mm.tile([P, NQS, NS], F32, tag="ops")
                    for ws in range(NQS):
                        for hh in range(H):
                            nc.tensor.matmul(
                                ops[:, ws, :], wo[:, hh, :], o_w[ws][:, hh, :],
                                start=(hh == 0), stop=(hh == H - 1))
                    nc.vector.tensor_tensor(
                        x2[:, dct, :],
                        ops.rearrange("p a b -> p (a b)"),
                        xq_sb[:, dct, :], ALU.add)
                    sq2 = p3.tile([P, QT], F16, tag="sq2")
                    nc.scalar.activation(sq2[:], x2[:, dct, :], AF.Square)
                    for ws in range(NQS):
                        nc.tensor.matmul(ssp2[:, ws * NS:(ws + 1) * NS],
                                         ones_c[:],
                                         sq2[:, ws * NS:(ws + 1) * NS],
                                         start=(dct == 0),
                                         stop=(dct == NDCT - 1))
                for ws in range(NQS):
                    q0 = ws * NS
                    srow2 = p3.tile([1, NS], F32, tag="srow2")
                    nc.scalar.activation(srow2[:], ssp2[:, q0:q0 + NS],
                                         AF.Sqrt, scale=1.0 / D, bias=epsT[:])
                    rec2 = p3.tile([1, NS], F16, tag="rec2")
                    nc.vector.reciprocal(rec2[:], srow2[:])
                    bc2 = p3bc.tile([P, NS], F32, tag="bc2")
                    nc.tensor.matmul(bc2[:], ones_r[:], rec2[:],
                                     start=True, stop=True)
                    bc2s = p3.tile([P, NS], F16, tag="bc2s")
                    nc.scalar.copy(bc2s[:], bc2[:])
                    for dt in range(DT):
                        nc.vector.tensor_tensor(h2[:, dt, q0:q0 + NS],
                                                x2[:, dt, q0:q0 + NS],
                                                bc2s[:], ALU.mult)
        po.release()
        pax.release()

        # ================= P5: SwiGLU MLP + residual =====================
        with nc.named_scope("P5"):
            with tc.tile_pool(name="p5", bufs=2) as p5, \
                 tc.tile_pool(name="p5w", bufs=2) as p5w, \
                 tc.tile_pool(name="p5mt", bufs=1) as p5mt, \
                 tc.tile_pool(name="p5gu", bufs=2, space="PSUM") as p5gu, \
                 tc.tile_pool(name="p5d", bufs=3, space="PSUM") as p5d:
                for ws in range(NQS):
                    q0 = ws * NS
                    mt = p5mt.tile([P, NFT, NS], F16, tag="mt")
                    for ft in range(NFT):
                        wg = p5w.tile([P, DT, P], F16, tag="wg")
                        nc.sync.dma_start(out=wg[:], in_=wg_in[ft])
                        wu = p5w.tile([P, DT, P], F16, tag="wu")
                        nc.sync.dma_start(out=wu[:], in_=wu_in[ft])
                        guw = p5gu.tile([P, 2, NS], F32, tag="guw")
                        for dt in range(DT):
                            nc.tensor.matmul(
                                guw[:, 0, :], wg[:, dt, :],
                                h2[:, dt, q0:q0 + NS],
                                start=(dt == 0), stop=(dt == DT - 1))
                        for dt in range(DT):
                            nc.tensor.matmul(
                                guw[:, 1, :], wu[:, dt, :],
                                h2[:, dt, q0:q0 + NS],
                                start=(dt == 0), stop=(dt == DT - 1))
                        sg = p5.tile([P, NS], F16, tag="sg")
                        nc.scalar.activation(sg[:], guw[:, 0, :], AF.Silu)
                        nc.vector.tensor_tensor(mt[:, ft, :], sg[:],
                                                guw[:, 1, :], ALU.mult)
                    for dct in range(NDCT):
                        wd = p5w.tile([P, NFT, P], F16, tag="wd")
                        nc.sync.dma_start(out=wd[:], in_=wd_in[dct])
                        dps = p5d.tile([P, NS], F32, tag="dacc")
                        for ft in range(NFT):
                            nc.tensor.matmul(dps[:], wd[:, ft, :],
                                             mt[:, ft, :],
                                             start=(ft == 0),
                                             stop=(ft == NFT - 1))
                        yt = p5.tile([P, NS], F32, tag="yt")
                        nc.vector.tensor_tensor(yt[:], dps[:],
                                                x2[:, dct, q0:q0 + NS],
                                                ALU.add)
                        nc.sync.dma_start(out=y_out[dct][:, q0:q0 + NS],
                                          in_=yt[:])
        px2.release()
        pc.release()

    nc.compile()
    return nc


# --------------------------------------------------------------------------
# Host side
# --------------------------------------------------------------------------

_NC_CACHE = {}


def _get_nc(cfg):
    key = (cfg.D, cfg.T, cfg.FF, cfg.QT)
    if key not in _NC_CACHE:
        _NC_CACHE[key] = build(cfg)
    return _NC_CACHE[key]


def _tile_lhs(a, ncols):
    # [Din, Cout] -> [Cout/ncols, P, Din/P, ncols]
    d, c = a.shape
    return np.ascontiguousarray(
        a.reshape(d // P, P, c // ncols, ncols).transpose(2, 1, 0, 3))


def prep_weights(cfg, w_qkv, w_out, w_gate, w_up, w_down, ln1, ln2):
    D, TD, FF, NS = cfg.D, cfg.TD, cfg.FF, cfg.NS
    f32, f16 = np.float32, np.float16
    w_qkv_f = (np.asarray(w_qkv, f32) * np.asarray(ln1, f32)[None, :])
    wqT = w_qkv_f[0:TD].T
    wkT = w_qkv_f[TD:2 * TD].T
    wvT = w_qkv_f[2 * TD:3 * TD].T
    woT = np.asarray(w_out, f32).T            # [TD, D]
    wgT = (np.asarray(w_gate, f32) * np.asarray(ln2, f32)[None, :]).T
    wuT = (np.asarray(w_up, f32) * np.asarray(ln2, f32)[None, :]).T
    wdT = np.asarray(w_down, f32).T           # [FF, D]

    wd_in = np.ascontiguousarray(
        wdT.reshape(cfg.NFT, P, cfg.NDCT, P).transpose(2, 1, 0, 3))
    return dict(
        wq_in=_tile_lhs(wqT, P).astype(f16),
        wk_in=_tile_lhs(wkT, P).astype(f16),
        wv_in=_tile_lhs(wvT, NS).astype(f16),
        wo_in=_tile_lhs(woT, P).astype(f16),
        wg_in=_tile_lhs(wgT, P).astype(f16),
        wu_in=_tile_lhs(wuT, P).astype(f16),
        wd_in=wd_in.astype(f16),
    )


def prep_core_inputs(cfg, xb, parity, wdict):
    """Per-core tensors for batch slice xb [T, D]; query tokens are the
    interleaved slice parity::stride."""
    T, D, QT, NS = cfg.T, cfg.D, cfg.QT, cfg.NS
    stride = cfg.stride
    f16 = np.float16
    xT = np.ascontiguousarray(np.asarray(xb, np.float32).T)   # [D, T]
    x_in = xT.reshape(cfg.DT, P, T).astype(f16)
    xq_in = np.ascontiguousarray(
        xT[:, parity::stride]).reshape(cfg.DT, P, QT).astype(f16)
    # 0/1 mask for the NSLOT diagonal k tiles of every query slice:
    # slot s, row ki, col q allowed iff 128*s + ki <= stride*q + parity
    ki = np.arange(P)[:, None, None]
    s = np.arange(cfg.NSLOT)[None, :, None]
    q = np.arange(NS)[None, None, :]
    m01_in = ((P * s + ki) <= (stride * q + parity)).astype(f16)
    out = dict(x_in=x_in, xq_in=xq_in, m01_in=m01_in)
    out.update(wdict)
    return out


def run(cfg, x, w_qkv, w_out, w_gate, w_up, w_down, ln1, ln2):
    nc = _get_nc(cfg)
    wdict = prep_weights(cfg, w_qkv, w_out, w_gate, w_up, w_down, ln1, ln2)
    x = np.asarray(x, np.float32)
    Bc = x.shape[0]
    in_maps = []
    for c in range(N_CORES):
        b, half = divmod(c, 2)
        in_maps.append(prep_core_inputs(cfg, x[b % Bc], half, wdict))
    res = run_bass_kernel_spmd(nc, in_maps, list(range(N_CORES)))
    y = np.empty((Bc, cfg.T, cfg.D), np.float32)
    for c in range(N_CORES):
        b, parity = divmod(c, 2)
        if b < Bc:
            yc = res.results[c]["y_out"].reshape(cfg.D, cfg.QT)
            y[b, parity::cfg.stride, :] = yc.T
    return y


def kernel(x, w_qkv, w_out, w_gate, w_up, w_down, ln1, ln2):
    return run(FULL, x, w_qkv, w_out, w_gate, w_up, w_down, ln1, ln2)


# revision 29
# speedup vs baseline: 1.1967x; 1.0238x over previous
"""Trainium2 Bass kernel for a dense transformer block (RMSNorm -> causal MHA
-> residual -> RMSNorm -> SwiGLU MLP -> residual), distributed over 8
NeuronCores with zero collectives.

Sharding: core c handles batch b = c//2 and query parity half = c%2 (the
interleaved token slice half::2, QT=1024 query tokens per core).  Each core
computes K/V for its whole batch; queries / out-proj / MLP only for its
1024 tokens.

This version runs all matmul operands in fp16 (full PE speed at any tile
size, half the DMA + SBUF of fp32), loads every weight exactly once
(weight-stationary loops over resident normalized activations), keeps the
softmax denominator off the PE (DVE accumulation of exp tiles), applies the
causal mask as a 0/1 fp16 multiply only on diagonal tiles (translation
invariance -> 8 distinct mask tiles), and uses 2-bank-wide PSUM tiles so
Act-engine instructions amortize their access latency.
"""

import numpy as np

import concourse.bass as bass
import concourse.bass_isa as bass_isa
import concourse.bacc as bacc
import concourse.mybir as mybir
from concourse.tile import TileContext
from concourse.bass_utils import run_bass_kernel_spmd

F32 = mybir.dt.float32
F16 = mybir.dt.float16
AF = mybir.ActivationFunctionType
ALU = mybir.AluOpType

P = 128
N_CORES = 8
EPS = 1e-6


class CFG:
    def __init__(self, D, T, FF, QT):
        self.D, self.T, self.TD, self.FF, self.QT = D, T, D, FF, QT
        self.NS = 512
        self.DT = D // P            # contraction tiles over model dim
        self.H = self.TD // P       # heads (dh == P)
        self.KT = T // P            # key tiles
        self.NB = T // self.NS      # 512-token blocks over full sequence
        self.NQS = QT // self.NS    # query slices
        self.NVS = self.TD // self.NS  # v column slabs
        self.NFT = FF // P          # ff tiles
        self.NDCT = D // P          # output col tiles
        self.stride = T // QT       # query interleave stride
        self.NSLOT = self.stride * self.NS // P  # partial (diagonal) k tiles / slice
        self.ISQ = 1.0 / float(np.sqrt(P))

    def nkt(self, qs):
        return min((qs + 1) * self.stride * self.NS // P, self.KT)

    def kt0(self, qs):
        # first partially-masked k tile for query slice qs
        return self.stride * qs * self.NS // P


FULL = CFG(D=2048, T=2048, FF=8192, QT=1024)


def build(cfg):
    D, T, TD, FF, QT, NS = cfg.D, cfg.T, cfg.TD, cfg.FF, cfg.QT, cfg.NS
    DT, H, KT, NB, NQS = cfg.DT, cfg.H, cfg.KT, cfg.NB, cfg.NQS
    NVS, NFT, NDCT, NSLOT = cfg.NVS, cfg.NFT, cfg.NDCT, cfg.NSLOT
    HPS = NS // P               # heads per v slab
    KG = max(T // 1024, 1)      # 1024-token groups for K projection
    stride = cfg.stride

    nc = bacc.Bacc("TRN2", target_bir_lowering=False, num_devices=N_CORES)

    # ---- inputs (pre-tiled on host, fp16) ----
    x_in = nc.dram_tensor("x_in", [DT, P, T], F16, kind="ExternalInput")
    xq_in = nc.dram_tensor("xq_in", [DT, P, QT], F16, kind="ExternalInput")
    m01_in = nc.dram_tensor("m01_in", [P, NSLOT, NS], F16, kind="ExternalInput")
    wq_in = nc.dram_tensor("wq_in", [H, P, DT, P], F16, kind="ExternalInput")
    wk_in = nc.dram_tensor("wk_in", [H, P, DT, P], F16, kind="ExternalInput")
    wv_in = nc.dram_tensor("wv_in", [NVS, P, DT, NS], F16, kind="ExternalInput")
    wo_in = nc.dram_tensor("wo_in", [NDCT, P, H, P], F16, kind="ExternalInput")
    wg_in = nc.dram_tensor("wg_in", [NFT, P, DT, P], F16, kind="ExternalInput")
    wu_in = nc.dram_tensor("wu_in", [NFT, P, DT, P], F16, kind="ExternalInput")
    wd_in = nc.dram_tensor("wd_in", [NDCT, P, NFT, P], F16, kind="ExternalInput")
    y_out = nc.dram_tensor("y_out", [NDCT, P, QT], F32, kind="ExternalOutput")

    # ---- scratch DRAM (K/V spill, fp16) ----
    k_d = nc.dram_tensor("k_d", [H, P, T], F16)
    v_d = nc.dram_tensor("v_d", [NVS, KT, P, NS], F16)

    with TileContext(nc) as tc, \
            nc.allow_low_precision("fp16 softmax/norm sums; tol 2e-2"):
        pc = tc.alloc_tile_pool(name="const", bufs=1)
        ones_c = pc.tile([P, 1], F16, tag="ones_c")
        nc.vector.memset(ones_c[:], 1.0)
        ones_r = pc.tile([1, P], F16, tag="ones_r")
        nc.vector.memset(ones_r[:], 1.0)
        epsT = pc.tile([1, 1], F32, tag="eps")
        nc.vector.memset(epsT[:], EPS)
        m01 = pc.tile([P, NSLOT, NS], F16, tag="m01")
        nc.sync.dma_start(out=m01[:], in_=m01_in[:])

        # persistent activations
        pax = tc.alloc_tile_pool(name="ax", bufs=1)      # hq + xq (A -> P3)
        xq_sb = pax.tile([P, DT, QT], F16, tag="xq")
        nc.sync.dma_start(
            out=xq_sb[:], in_=xq_in.rearrange("a p c -> p a c"))
        hq = pax.tile([P, DT, QT], F16, tag="hq")

        # ========== A+B: rmsnorm + h, interleaved with K projection ======
        # The K pass for block tb-1 is emitted right after block tb's norm
        # chain, so the PE chews 55us of K matmuls while the next block's
        # DMA/Square/rsqrt pipeline runs on the other engines.
        ph = tc.alloc_tile_pool(name="h", bufs=1)        # h blocks (A -> B)
        h_blk = [ph.tile([P, DT, NS], F16, tag=f"h{tb}", name=f"h{tb}")
                 for tb in range(NB)]
        with nc.named_scope("AB"):
            with tc.tile_pool(name="pa", bufs=2) as pa, \
                 tc.tile_pool(name="pb", bufs=3) as pb, \
                 tc.tile_pool(name="pbk", bufs=2) as pbk, \
                 tc.tile_pool(name="pa_ps", bufs=2, space="PSUM") as pa_ps, \
                 tc.tile_pool(name="pa_bc", bufs=2, space="PSUM") as pa_bc, \
                 tc.tile_pool(name="pb_k", bufs=3, space="PSUM") as pb_k:

                def norm_into(dst, src, t0):
                    """rmsnorm scale of 512 tokens of src -> dst (both
                    [P, DT, *] views at column t0)."""
                    ssp = pa_ps.tile([1, NS], F32, tag="ssp")
                    for c in range(DT // 4):
                        sq = pa.tile([P, 4, NS], F16, tag="sq")
                        nc.scalar.activation(
                            sq[:], src[:, 4 * c:4 * c + 4, t0:t0 + NS],
                            AF.Square)
                        for j in range(4):
                            dt = 4 * c + j
                            nc.tensor.matmul(ssp[:], ones_c[:], sq[:, j, :],
                                             start=(dt == 0),
                                             stop=(dt == DT - 1))
                    srow = pa.tile([1, NS], F32, tag="srow")
                    nc.scalar.activation(srow[:], ssp[:], AF.Sqrt,
                                         scale=1.0 / D, bias=epsT[:])
                    rec32 = pa.tile([1, NS], F32, tag="rec32")
                    nc.vector.reciprocal_approx_fast(rec32[:], srow[:])
                    rec16 = pa.tile([1, NS], F16, tag="rec16")
                    nc.scalar.copy(rec16[:], rec32[:])
                    bcp = pa_bc.tile([P, NS], F32, tag="bcp")
                    nc.tensor.matmul(bcp[:], ones_r[:], rec16[:],
                                     start=True, stop=True)
                    bcs = pa.tile([P, NS], F16, tag="bcs")
                    nc.scalar.copy(bcs[:], bcp[:])
                    for dt in range(DT):
                        nc.vector.tensor_tensor(dst[:, dt, t0:t0 + NS],
                                                src[:, dt, t0:t0 + NS],
                                                bcs[:], ALU.mult)

                def load_xb(tb):
                    xb = pa.tile([P, DT, NS], F16, tag="xb", name=f"xb{tb}")
                    nc.sync.dma_start(
                        out=xb[:],
                        in_=x_in[:, :, tb * NS:(tb + 1) * NS].rearrange(
                            "a p c -> p a c"))
                    return xb

                def k_pass(sl):
                    for hh in range(H):
                        wk = pbk.tile([P, DT, P], F16, tag="wk")
                        nc.sync.dma_start(out=wk[:], in_=wk_in[hh])
                        kps = pb_k.tile([P, NS], F32, tag="kps")
                        for dt in range(DT):
                            nc.tensor.matmul(
                                kps[:], wk[:, dt, :], h_blk[sl][:, dt, :],
                                start=(dt == 0), stop=(dt == DT - 1))
                        kcp = pb.tile([P, NS], F16, tag="kcp")
                        nc.scalar.copy(kcp[:], kps[:])
                        nc.sync.dma_start(
                            out=k_d[hh][:, sl * NS:(sl + 1) * NS],
                            in_=kcp[:])

                xb_cur = load_xb(0)
                for tb in range(NB):
                    xb_next = load_xb(tb + 1) if tb + 1 < NB else None
                    norm_into(h_blk[tb], xb_cur, 0)
                    if tb >= 1:
                        k_pass(tb - 1)
                    xb_cur = xb_next
                # hq = xq * rsqrt: recompute stats from the (host-sliced)
                # query tokens — keeps the program parity-independent.
                for ws in range(NQS):
                    norm_into(hq, xq_sb, ws * NS)
                k_pass(NB - 1)

            # ---- V projection (h complete by now) ----
            with tc.tile_pool(name="pbv2", bufs=3) as pb2, \
                 tc.tile_pool(name="pbw", bufs=2) as pbw, \
                 tc.tile_pool(name="pb_v", bufs=3, space="PSUM") as pb_v:
                for vs in range(NVS):
                    wv = pbw.tile([P, DT, NS], F16, tag="wv")
                    nc.sync.dma_start(out=wv[:], in_=wv_in[vs])
                    for kt in range(KT):
                        tb, off = divmod(kt * P, NS)
                        vps = pb_v.tile([P, NS], F32, tag="vps")
                        for dt in range(DT):
                            nc.tensor.matmul(
                                vps[:], h_blk[tb][:, dt, off:off + P],
                                wv[:, dt, :],
                                start=(dt == 0), stop=(dt == DT - 1))
                        vcp = pb2.tile([P, NS], F16, tag="vcp")
                        nc.scalar.copy(vcp[:], vps[:])
                        nc.sync.dma_start(out=v_d[vs, kt], in_=vcp[:])
        ph.release()

        # ================= P2: Q projection + causal attention ===========
        po = tc.alloc_tile_pool(name="o", bufs=1)        # o (P2 -> P3)
        o_w = [po.tile([P, H, NS], F16, tag=f"o{ws}", name=f"o{ws}")
               for ws in range(NQS)]
        with nc.named_scope("P2"):
            with tc.tile_pool(name="p2", bufs=2) as p2, \
                 tc.tile_pool(name="p2kv", bufs=2) as p2kv, \
                 tc.tile_pool(name="p2pex", bufs=4) as p2pex, \
                 tc.tile_pool(name="p2w", bufs=2) as p2w, \
                 tc.tile_pool(name="p2mm", bufs=3, space="PSUM") as p2mm, \
                 tc.tile_pool(name="p2acc", bufs=2, space="PSUM") as p2acc:
                for hh in range(H):
                    kh = p2kv.tile([P, T], F16, tag="kh")
                    nc.sync.dma_start(out=kh[:], in_=k_d[hh])
                    vh = p2kv.tile([P, KT, P], F16, tag="vh")
                    voff = (hh % HPS) * P
                    nc.sync.dma_start(
                        out=vh[:],
                        in_=v_d[hh // HPS].rearrange(
                            "t p c -> p t c")[:, :, voff:voff + P])
                    # --- Q projection for this head (fills PE while Act
                    #     runs exp for the previous head) ---
                    wq = p2w.tile([P, DT, P], F16, tag="wq")
                    nc.sync.dma_start(out=wq[:], in_=wq_in[hh])
                    qps = p2mm.tile([P, NQS * NS], F32, tag="mm")
                    for ws in range(NQS):
                        for dt in range(DT):
                            nc.tensor.matmul(
                                qps[:, ws * NS:(ws + 1) * NS],
                                wq[:, dt, :], hq[:, dt, ws * NS:(ws + 1) * NS],
                                start=(dt == 0), stop=(dt == DT - 1))
                    qh = p2.tile([P, QT], F16, tag="qh")
                    nc.scalar.copy(qh[:], qps[:])
                    for qs in range(NQS):
                        nkt = cfg.nkt(qs)
                        kt0 = cfg.kt0(qs)
                        npair = nkt // 2
                        oacc = p2acc.tile([P, NS], F32, tag="oacc")
                        dsum = p2.tile([P, NS], F16, tag="dsum")
                        scps = {}

                        def emit_scp(kp):
                            scp = p2mm.tile([P, 2, NS], F32, tag="mm")
                            for half in range(2):
                                kt = 2 * kp + half
                                nc.tensor.matmul(
                                    scp[:, half, :], kh[:, kt * P:(kt + 1) * P],
                                    qh[:, qs * NS:(qs + 1) * NS],
                                    start=True, stop=True)
                            scps[kp] = scp

                        def emit_rest(kp):
                            scp = scps.pop(kp)
                            pex = p2pex.tile([P, 2, NS], F16, tag="pex")
                            nc.scalar.activation(pex[:], scp[:], AF.Exp,
                                                 scale=cfg.ISQ)
                            if 2 * kp >= kt0:
                                s = 2 * kp - kt0
                                pexm = p2pex.tile([P, 2, NS], F16, tag="pexm")
                                nc.vector.tensor_tensor(
                                    pexm[:], pex[:], m01[:, s:s + 2, :],
                                    ALU.mult)
                                pex = pexm
                            if kp == 0:
                                nc.vector.tensor_tensor(
                                    dsum[:], pex[:, 0, :], pex[:, 1, :],
                                    ALU.add)
                            else:
                                for half in range(2):
                                    nc.vector.tensor_tensor(
                                        dsum[:], dsum[:], pex[:, half, :],
                                        ALU.add)
                            for half in range(2):
                                kt = 2 * kp + half
                                nc.tensor.matmul(
                                    oacc[:], vh[:, kt, :], pex[:, half, :],
                                    start=(kt == 0), stop=(kt == nkt - 1))

                        emit_scp(0)
                        if npair > 1:
                            emit_scp(1)
                        for kp in range(npair):
                            if kp + 2 < npair:
                                emit_scp(kp + 2)
                            emit_rest(kp)
                        # softmax denominator (Pool engine: cross-partition
                        # sum broadcast to every partition) -> normalize
                        dbc = p2.tile([P, NS], F32, tag="dbc")
                        nc.gpsimd.partition_all_reduce(
                            dbc[:], dsum[:], P, bass_isa.ReduceOp.add)
                        recb = p2.tile([P, NS], F32, tag="recb")
                        nc.vector.reciprocal_approx_fast(recb[:], dbc[:])
                        nc.vector.tensor_tensor(o_w[qs][:, hh, :], oacc[:],
                                                recb[:], ALU.mult)

        # ================= P3: out-proj + residual + norm2 ===============
        # right-side stack: lifetime (P3 -> P5) crosses po's release
        px2 = tc.alloc_tile_pool(name="x2h2", bufs=1, side="right")
        x2 = px2.tile([P, NDCT, QT], F16, tag="x2")
        h2 = px2.tile([P, DT, QT], F16, tag="h2")
        with nc.named_scope("P3"):
            with tc.tile_pool(name="p3", bufs=2) as p3, \
                 tc.tile_pool(name="p3w", bufs=2) as p3w, \
                 tc.tile_pool(name="p3mm", bufs=2, space="PSUM") as p3mm, \
                 tc.tile_pool(name="p3s", bufs=1, space="PSUM") as p3s, \
                 tc.tile_pool(name="p3bc", bufs=2, space="PSUM") as p3bc:
                ssp2 = p3s.tile([1, QT], F32, tag="ssp2")

                def stat2(dct, sq2):
                    for ws in range(NQS):
                        nc.tensor.matmul(ssp2[:, ws * NS:(ws + 1) * NS],
                                         ones_c[:],
                                         sq2[:, ws * NS:(ws + 1) * NS],
                                         start=(dct == 0),
                                         stop=(dct == NDCT - 1))

                pend = []
                for dct in range(NDCT):
                    wo = p3w.tile([P, H, P], F16, tag="wo")
                    nc.sync.dma_start(out=wo[:], in_=wo_in[dct])
                    ops = p3mm.tile([P, NQS, NS], F32, tag="ops")
                    for ws in range(NQS):
                        for hh in range(H):
                            nc.tensor.matmul(
                                ops[:, ws, :], wo[:, hh, :], o_w[ws][:, hh, :],
                                start=(hh == 0), stop=(hh == H - 1))
                    nc.vector.tensor_tensor(
                        x2[:, dct, :],
                        ops.rearrange("p a b -> p (a b)"),
                        xq_sb[:, dct, :], ALU.add)
                    sq2 = p3.tile([P, QT], F16, tag="sq2", bufs=4)
                    nc.scalar.activation(sq2[:], x2[:, dct, :], AF.Square)
                    # delay the tiny stat matmuls two dcts so the PE isn't
                    # held hostage to the DVE->Act chain of the current dct
                    pend.append((dct, sq2))
                    if len(pend) >= 3:
                        stat2(*pend.pop(0))
                for it in pend:
                    stat2(*it)
                for ws in range(NQS):
                    q0 = ws * NS
                    srow2 = p3.tile([1, NS], F32, tag="srow2")
                    nc.scalar.activation(srow2[:], ssp2[:, q0:q0 + NS],
                                         AF.Sqrt, scale=1.0 / D, bias=epsT[:])
                    rec232 = p3.tile([1, NS], F32, tag="rec232")
                    nc.vector.reciprocal_approx_fast(rec232[:], srow2[:])
                    rec2 = p3.tile([1, NS], F16, tag="rec2")
                    nc.scalar.copy(rec2[:], rec232[:])
                    bc2 = p3bc.tile([P, NS], F32, tag="bc2")
                    nc.tensor.matmul(bc2[:], ones_r[:], rec2[:],
                                     start=True, stop=True)
                    bc2s = p3.tile([P, NS], F16, tag="bc2s")
                    nc.scalar.copy(bc2s[:], bc2[:])
                    for dt in range(DT):
                        nc.vector.tensor_tensor(h2[:, dt, q0:q0 + NS],
                                                x2[:, dt, q0:q0 + NS],
                                                bc2s[:], ALU.mult)
        po.release()
        pax.release()

        # ================= P5: SwiGLU MLP + residual =====================
        with nc.named_scope("P5"):
            with tc.tile_pool(name="p5", bufs=2) as p5, \
                 tc.tile_pool(name="p5w", bufs=2) as p5w, \
                 tc.tile_pool(name="p5mt", bufs=1) as p5mt, \
                 tc.tile_pool(name="p5gu", bufs=2, space="PSUM") as p5gu, \
                 tc.tile_pool(name="p5d", bufs=3, space="PSUM") as p5d:
                for ws in range(NQS):
                    q0 = ws * NS
                    mt = p5mt.tile([P, NFT, NS], F16, tag="mt")
                    for ft in range(NFT):
                        wg = p5w.tile([P, DT, P], F16, tag="wg")
                        nc.sync.dma_start(out=wg[:], in_=wg_in[ft])
                        wu = p5w.tile([P, DT, P], F16, tag="wu")
                        nc.sync.dma_start(out=wu[:], in_=wu_in[ft])
                        guw = p5gu.tile([P, 2, NS], F32, tag="guw")
                        for dt in range(DT):
                            nc.tensor.matmul(
                                guw[:, 0, :], wg[:, dt, :],
                                h2[:, dt, q0:q0 + NS],
                                start=(dt == 0), stop=(dt == DT - 1))
                        for dt in range(DT):
                            nc.tensor.matmul(
                                guw[:, 1, :], wu[:, dt, :],
                                h2[:, dt, q0:q0 + NS],
                                start=(dt == 0), stop=(dt == DT - 1))
                        sg = p5.tile([P, NS], F16, tag="sg")
                        nc.scalar.activation(sg[:], guw[:, 0, :], AF.Silu)
                        nc.vector.tensor_tensor(mt[:, ft, :], sg[:],
                                                guw[:, 1, :], ALU.mult)
                    for dct in range(NDCT):
                        wd = p5w.tile([P, NFT, P], F16, tag="wd")
                        nc.sync.dma_start(out=wd[:], in_=wd_in[dct])
                        dps = p5d.tile([P, NS], F32, tag="dacc")
                        for ft in range(NFT):
                            nc.tensor.matmul(dps[:], wd[:, ft, :],
                                             mt[:, ft, :],
                                             start=(ft == 0),
                                             stop=(ft == NFT - 1))
                        yt = p5.tile([P, NS], F32, tag="yt")
                        nc.vector.tensor_tensor(yt[:], dps[:],
                                                x2[:, dct, q0:q0 + NS],
                                                ALU.add)
                        nc.sync.dma_start(out=y_out[dct][:, q0:q0 + NS],
                                          in_=yt[:])
        px2.release()
        pc.release()

    nc.compile()
    return nc


# --------------------------------------------------------------------------
# Host side
# --------------------------------------------------------------------------

_NC_CACHE = {}


def _get_nc(cfg):
    key = (cfg.D, cfg.T, cfg.FF, cfg.QT)
    if key not in _NC_CACHE:
        _NC_CACHE[key] = build(cfg)
    return _NC_CACHE[key]


def _tile_lhs(a, ncols):
    # [Din, Cout] -> [Cout/ncols, P, Din/P, ncols]
    d, c = a.shape
    return np.ascontiguousarray(
        a.reshape(d // P, P, c // ncols, ncols).transpose(2, 1, 0, 3))


def prep_weights(cfg, w_qkv, w_out, w_gate, w_up, w_down, ln1, ln2):
    D, TD, FF, NS = cfg.D, cfg.TD, cfg.FF, cfg.NS
    f32, f16 = np.float32, np.float16
    w_qkv_f = (np.asarray(w_qkv, f32) * np.asarray(ln1, f32)[None, :])
    wqT = w_qkv_f[0:TD].T
    wkT = w_qkv_f[TD:2 * TD].T
    wvT = w_qkv_f[2 * TD:3 * TD].T
    woT = np.asarray(w_out, f32).T            # [TD, D]
    wgT = (np.asarray(w_gate, f32) * np.asarray(ln2, f32)[None, :]).T
    wuT = (np.asarray(w_up, f32) * np.asarray(ln2, f32)[None, :]).T
    wdT = np.asarray(w_down, f32).T           # [FF, D]

    wd_in = np.ascontiguousarray(
        wdT.reshape(cfg.NFT, P, cfg.NDCT, P).transpose(2, 1, 0, 3))
    return dict(
        wq_in=_tile_lhs(wqT, P).astype(f16),
        wk_in=_tile_lhs(wkT, P).astype(f16),
        wv_in=_tile_lhs(wvT, NS).astype(f16),
        wo_in=_tile_lhs(woT, P).astype(f16),
        wg_in=_tile_lhs(wgT, P).astype(f16),
        wu_in=_tile_lhs(wuT, P).astype(f16),
        wd_in=wd_in.astype(f16),
    )


def prep_core_inputs(cfg, xb, parity, wdict):
    """Per-core tensors for batch slice xb [T, D]; query tokens are the
    interleaved slice parity::stride."""
    T, D, QT, NS = cfg.T, cfg.D, cfg.QT, cfg.NS
    stride = cfg.stride
    f16 = np.float16
    xT = np.ascontiguousarray(np.asarray(xb, np.float32).T)   # [D, T]
    x_in = xT.reshape(cfg.DT, P, T).astype(f16)
    xq_in = np.ascontiguousarray(
        xT[:, parity::stride]).reshape(cfg.DT, P, QT).astype(f16)
    # 0/1 mask for the NSLOT diagonal k tiles of every query slice:
    # slot s, row ki, col q allowed iff 128*s + ki <= stride*q + parity
    ki = np.arange(P)[:, None, None]
    s = np.arange(cfg.NSLOT)[None, :, None]
    q = np.arange(NS)[None, None, :]
    m01_in = ((P * s + ki) <= (stride * q + parity)).astype(f16)
    out = dict(x_in=x_in, xq_in=xq_in, m01_in=m01_in)
    out.update(wdict)
    return out


def run(cfg, x, w_qkv, w_out, w_gate, w_up, w_down, ln1, ln2):
    nc = _get_nc(cfg)
    wdict = prep_weights(cfg, w_qkv, w_out, w_gate, w_up, w_down, ln1, ln2)
    x = np.asarray(x, np.float32)
    Bc = x.shape[0]
    in_maps = []
    for c in range(N_CORES):
        b, half = divmod(c, 2)
        in_maps.append(prep_core_inputs(cfg, x[b % Bc], half, wdict))
    res = run_bass_kernel_spmd(nc, in_maps, list(range(N_CORES)))
    y = np.empty((Bc, cfg.T, cfg.D), np.float32)
    for c in range(N_CORES):
        b, parity = divmod(c, 2)
        if b < Bc:
            yc = res.results[c]["y_out"].reshape(cfg.D, cfg.QT)
            y[b, parity::cfg.stride, :] = yc.T
    return y


def kernel(x, w_qkv, w_out, w_gate, w_up, w_down, ln1, ln2):
    return run(FULL, x, w_qkv, w_out, w_gate, w_up, w_down, ln1, ln2)
